# revision 7
# baseline (speedup 1.0000x reference)
"""Trainium2 Bass kernel for nn_Attention (B=16,N=4096,C=1024,H=16,HD=64,Q=64).

Data-parallel over B across 8 NeuronCores (2 batches/core). Per batch the
attention is reassociated so no k/v tensors are materialized and no on-chip
transposes are needed:

  q^T = Wq @ x_q^T                      [(h,d)=1024, 64]
  G_h^T = Wk_h^T @ q_h                  G^T: [c=1024, (h,q)=1024]
  S^T   = x @ G^T   (per t-tile)        [t, (h,q)]   (contract c)
  p^T   = exp(S^T / 8)                  (softmax w/o max-sub: scores ~ +-5)
  u^T   = x^T(nat) @ p^T  (accum t)     [c, (h,q)]   (contract t)
  den   = ones @ pacc     (pacc: GpSimd p-sum over t)
  o_h^T = (Wv_h^T)^T @ u_h^T, scaled by 1/den at PSUM eviction
  y     = o^T.T @ Wproj^T + b           [64, 1024]   (contract (h,d))

Both batches run as one flat 16-block pipeline: u-rounds are delayed one
block behind the S/exp stream so the scalar-engine exp always has a full
block of slack, and the q/G prologue + o/proj epilogue batch the two
batches through shared stationary weights (moving dim 128/256 instead of
64).  Epilogue-only weights (Wv/Wproj) are DMAed mid-t-loop so the
prologue weights get full HBM bandwidth.  The u accumulator is bf16 in
SBUF (both batches in one tile) so o's moving operand can span batches
with a strided AP.
"""
import numpy as np

B, N, C = 16, 4096, 1024
H, HD, QL = 16, 64, 64
BL = B // 8           # batches per core
CK = C // 128         # 8 c-tiles
TB = 512              # tokens per t-block
NBLK = N // TB        # 8 blocks per batch
TPB = TB // 128       # 4 t-tiles per block
VB = BL * NBLK        # 16 virtual blocks
HQ = H * QL           # 1024
SCALE = HD ** -0.5

_CACHE = {}


def _build():
    import concourse.bass as bass
    import concourse.tile as tile
    from concourse import bacc, mybir

    f32 = mybir.dt.float32
    bf16 = mybir.dt.bfloat16
    EXP = mybir.ActivationFunctionType.Exp

    nc = bacc.Bacc("TRN2", target_bir_lowering=False, debug=False, num_devices=8)
    xn = nc.dram_tensor("xn", [BL, N, C], bf16, kind="ExternalInput").ap()
    xt = nc.dram_tensor("xt", [BL, C, N], bf16, kind="ExternalInput").ap()
    wq = nc.dram_tensor("wq", [C, C], bf16, kind="ExternalInput").ap()   # Wq^T
    wk = nc.dram_tensor("wk", [C, C], bf16, kind="ExternalInput").ap()   # Wk natural
    wv = nc.dram_tensor("wv", [C, C], bf16, kind="ExternalInput").ap()   # Wv^T
    wp = nc.dram_tensor("wp", [C, C], bf16, kind="ExternalInput").ap()   # Wproj^T
    bp = nc.dram_tensor("bp", [1, C], f32, kind="ExternalInput").ap()
    xq = nc.dram_tensor("xq", [BL, C, QL], bf16, kind="ExternalInput").ap()
    y = nc.dram_tensor("y", [BL, QL, C], f32, kind="ExternalOutput").ap()

    with tile.TileContext(nc) as tc:
        with (
            tc.tile_pool(name="wpool", bufs=2) as wpool,
            tc.tile_pool(name="xtp", bufs=2) as xtp,
            tc.tile_pool(name="xnp", bufs=3) as xnp,
            tc.tile_pool(name="gpool", bufs=1) as gpool,
            tc.tile_pool(name="upool", bufs=1) as upool,
            tc.tile_pool(name="small", bufs=1) as small,
            tc.tile_pool(name="ptp", bufs=3) as ptp,
            tc.tile_pool(name="psa", bufs=4, space="PSUM") as psa,
            tc.tile_pool(name="psu", bufs=4, space="PSUM") as psu,
        ):
            ones32 = small.tile([128, 8], f32, tag="ones32")
            nc.gpsimd.memset(ones32[:], 1.0)
            bps = small.tile([128, C], bf16, tag="bps")
            nc.gpsimd.dma_start(bps[0:1, :], bp[:, :])
            bpf = small.tile([128, C], bf16, tag="bpf")
            nc.gpsimd.partition_broadcast(bpf[:], bps[0:1, :])

            # xq for both batches, batch-minor: [128, ck, (b0 64 | b1 64)]
            xqt = small.tile([128, CK, 128], bf16, tag="xqt", name="xqt")
            for b in range(BL):
                for ck in range(CK):
                    nc.sync.dma_start(xqt[:, ck, b * 64:(b + 1) * 64],
                                      xq[b, ck * 128:(ck + 1) * 128, :])

            # prologue weights, tiles interleaved so q's ck-loop starts early
            wt = wpool.tile([128, 8 * 1024], bf16, tag="w", name="wt_q")
            wt2 = wpool.tile([128, 8 * 1024], bf16, tag="w", name="wt_k")
            for ck in range(CK):
                nc.sync.dma_start(wt[:, ck * 1024:(ck + 1) * 1024],
                                  wq[ck * 128:(ck + 1) * 128, :])
                nc.sync.dma_start(wt2[:, ck * 1024:(ck + 1) * 1024],
                                  wk[ck * 128:(ck + 1) * 128, :])

            # ---------- q^T for both batches ----------
            # psum jc: [hd-of-pair(128), (b0 64q | b1 64q)]; lands in the
            # block-diagonal layout qbd[:, pair, (b 128: h0 64 | h1 64)]
            qbd = small.tile([128, 8, 256], bf16, tag="qbd", name="qbd")
            nc.gpsimd.memset(qbd[:], 0.0)
            for jc in range(8):
                ps = psa.tile([128, 512], f32, tag="psa")
                for ck in range(CK):
                    nc.tensor.matmul(
                        ps[:, 0:128],
                        wt[:, ck * 1024 + jc * 128: ck * 1024 + (jc + 1) * 128],
                        xqt[:, ck, :],
                        start=(ck == 0), stop=(ck == CK - 1))
                for b in range(BL):
                    for sub in range(2):
                        nc.vector.tensor_copy(
                            qbd[sub * 64:(sub + 1) * 64, jc,
                                b * 128 + sub * 64: b * 128 + (sub + 1) * 64],
                            ps[sub * 64:(sub + 1) * 64, b * 64:(b + 1) * 64])

            # ---------- G^T for both batches ----------
            # per (ck, pair): N=256 matmul; psum holds 2 pairs
            gts = [gpool.tile([128, CK * 1024], bf16, tag="gt", bufs=2,
                              name=f"gt{b}") for b in range(BL)]
            for ck in range(CK):
                for ph in range(4):
                    ps = psa.tile([128, 512], f32, tag="psa")
                    for k in range(2):
                        pair = ph * 2 + k
                        nc.tensor.matmul(
                            ps[:, k * 256:(k + 1) * 256],
                            wt2[:, pair * 1024 + ck * 128:
                                pair * 1024 + (ck + 1) * 128],
                            qbd[:, pair, :], start=True, stop=True)
                    for k in range(2):
                        pair = ph * 2 + k
                        for b in range(BL):
                            nc.vector.tensor_copy(
                                gts[b][:, ck * 1024 + pair * 128:
                                       ck * 1024 + (pair + 1) * 128],
                                ps[:, k * 256 + b * 128: k * 256 + (b + 1) * 128])

            # u accumulator (bf16, both batches)
            uacc = upool.tile([128, BL, CK * 1024], bf16, name="uacc")
            paccs = []
            for b in range(BL):
                pa = small.tile([128, HQ], f32, tag="pacc", bufs=2,
                                name=f"pacc{b}")
                nc.gpsimd.memset(pa[:], 0.0)
                paccs.append(pa)

            ptcs = {}
            xnts = {}
            rds = {}

            def emit_block(vb):
                b, blk = divmod(vb, NBLK)
                xtt = xtp.tile([128, CK * TB], bf16, tag="xt", name=f"xt{vb}")
                for ck in range(CK):
                    nc.sync.dma_start(
                        xtt[:, ck * TB:(ck + 1) * TB],
                        xt[b, ck * 128:(ck + 1) * 128, blk * TB:(blk + 1) * TB])
                xnt = xnp.tile([128, TPB * 1024], bf16, tag="xn", name=f"xn{vb}")
                for i in range(TPB):
                    nc.scalar.dma_start(
                        xnt[:, i * 1024:(i + 1) * 1024],
                        xn[b, (blk * TPB + i) * 128:(blk * TPB + i + 1) * 128, :])
                xnts[vb] = xnt
                ptc = ptp.tile([128, TPB * 1024], bf16, tag="ptc", name=f"ptc{vb}")
                for i in range(TPB):
                    for qh in range(2):
                        st = psa.tile([128, 512], f32, tag="psa")
                        for ck in range(CK):
                            nc.tensor.matmul(
                                st[:],
                                xtt[:, ck * TB + i * 128: ck * TB + (i + 1) * 128],
                                gts[b][:, ck * 1024 + qh * 512:
                                       ck * 1024 + (qh + 1) * 512],
                                start=(ck == 0), stop=(ck == CK - 1))
                        pslice = ptc[:, i * 1024 + qh * 512: i * 1024 + (qh + 1) * 512]
                        nc.scalar.activation(pslice, st[:], EXP, scale=SCALE)
                        pa = paccs[b][:, qh * 512:(qh + 1) * 512]
                        nc.gpsimd.tensor_add(pa, pslice, pa)
                ptcs[vb] = ptc

            def emit_uround(p, mid_cb=None):
                b, r = divmod(p, 4)
                v0, v1 = 2 * p, 2 * p + 1
                for qh in range(2):
                    if qh == 1 and mid_cb is not None:
                        mid_cb()
                    for cq in range(2):
                        ups = [psu.tile([128, 512], f32, tag="ups",
                                        name=f"ups{p}_{qh}_{cq}_{j}")
                               for j in range(4)]
                        for half, v in enumerate((v0, v1)):
                            pp, xx = ptcs[v], xnts[v]
                            for i in range(TPB):
                                for k4 in range(4):
                                    ck = cq * 4 + k4
                                    nc.tensor.matmul(
                                        ups[k4][:],
                                        xx[:, i * 1024 + ck * 128: i * 1024 + (ck + 1) * 128],
                                        pp[:, i * 1024 + qh * 512: i * 1024 + (qh + 1) * 512],
                                        start=(half == 0 and i == 0),
                                        stop=(half == 1 and i == TPB - 1))
                        for k4 in range(4):
                            ck = cq * 4 + k4
                            dst = uacc[:, b, ck * 1024 + qh * 512: ck * 1024 + (qh + 1) * 512]
                            if r == 0:
                                nc.vector.tensor_copy(dst, ups[k4][:])
                            else:
                                nc.vector.tensor_add(dst, ups[k4][:], dst)
                del ptcs[v0], ptcs[v1], xnts[v0], xnts[v1]

            def emit_den(b):
                rd = small.tile([128, HQ], f32, tag="rd", name=f"rd{b}")
                for qh in range(2):
                    dnp = psa.tile([128, 512], f32, tag="psa", name=f"dnp{b}_{qh}")
                    nc.tensor.matmul(dnp[0:8, :], ones32[:],
                                     paccs[b][:, qh * 512:(qh + 1) * 512],
                                     start=True, stop=True)
                    nc.vector.reciprocal(rd[0:1, qh * 512:(qh + 1) * 512],
                                         dnp[0:1, :])
                rdf = small.tile([128, HQ], f32, tag="rdf", name=f"rdf{b}")
                nc.gpsimd.partition_broadcast(rdf[:], rd[0:1, :])
                rds[b] = rdf

            # per-head-pair reciprocal layout for the o^T scale:
            # rdo[p, jc, b*64+qq] = 1/d_b[(2jc + p//64)*64 + qq]
            rdo = small.tile([128, 8, 128], f32, tag="rdo", name="rdo")

            def emit_rdo(b):
                rdf = rds[b]
                for jc in range(8):
                    nc.vector.tensor_copy(
                        rdo[0:64, jc, b * 64:(b + 1) * 64],
                        rdf[0:64, (2 * jc) * 64:(2 * jc + 1) * 64])
                    nc.vector.tensor_copy(
                        rdo[64:128, jc, b * 64:(b + 1) * 64],
                        rdf[64:128, (2 * jc + 1) * 64:(2 * jc + 2) * 64])

            # ---------- flat 16-block pipeline ----------
            wt3 = wt4 = None
            for vb in range(VB):
                emit_block(vb)
                if vb == 1:
                    # epilogue weights ride a slow ring mid-loop
                    wt3 = wpool.tile([128, 8 * 1024], bf16, tag="w", name="wt_v")
                    for ck in range(CK):
                        nc.gpsimd.dma_start(wt3[:, ck * 1024:(ck + 1) * 1024],
                                            wv[ck * 128:(ck + 1) * 128, :])
                if vb == 2:
                    wt4 = wpool.tile([128, 8 * 1024], bf16, tag="w", name="wt_p")
                    for jc in range(CK):
                        nc.gpsimd.dma_start(wt4[:, jc * 1024:(jc + 1) * 1024],
                                            wp[jc * 128:(jc + 1) * 128, :])
                if vb >= 2 and vb % 2 == 0:
                    p = vb // 2 - 1
                    if p == 3:
                        emit_uround(p, mid_cb=lambda: (emit_den(0), emit_rdo(0)))
                    else:
                        emit_uround(p)

            # last u-round; denominators for b1 slot into its qh gap
            emit_uround(7, mid_cb=lambda: (emit_den(1), emit_rdo(1)))

            # ---------- joint epilogue ----------
            # o^T per head pair, moving spans both batches via strided AP
            oT = small.tile([128, 8, 128], bf16, tag="oT", name="oT")
            for jc in range(8):
                ps = psa.tile([128, 512], f32, tag="psa")
                for sub in range(2):
                    h = jc * 2 + sub
                    ucol = (h // 8) * 512 + (h % 8) * 64
                    for ck in range(CK):
                        nc.tensor.matmul(
                            ps[sub * 64:(sub + 1) * 64, 0:128],
                            wt3[:, ck * 1024 + h * 64: ck * 1024 + (h + 1) * 64],
                            uacc[:, :, ck * 1024 + ucol: ck * 1024 + ucol + 64],
                            start=(ck == 0), stop=(ck == CK - 1),
                            tile_position=(0, sub * 64))
                nc.vector.tensor_mul(oT[:, jc, :], ps[:, 0:128], rdo[:, jc, :])

            for b in range(BL):
                ys = small.tile([128, C], f32, tag="ys", bufs=2, name=f"ys{b}")
                for half in range(2):
                    ps = psa.tile([128, 512], f32, tag="psa")
                    for jc in range(8):
                        nc.tensor.matmul(
                            ps[0:QL, :],
                            oT[:, jc, b * 64:(b + 1) * 64],
                            wt4[:, jc * 1024 + half * 512: jc * 1024 + (half + 1) * 512],
                            start=(jc == 0), stop=(jc == 7))
                    nc.vector.tensor_add(
                        ys[0:QL, half * 512:(half + 1) * 512], ps[0:QL, :],
                        bpf[0:QL, half * 512:(half + 1) * 512])
                nc.sync.dma_start(y[b, :, :], ys[0:QL, :])

    nc.compile()
    return nc


def get_nc():
    if "nc" not in _CACHE:
        _CACHE["nc"] = _build()
    return _CACHE["nc"]


def make_in_maps(x, Wq, Wk, Wv, Wproj, bproj):
    import ml_dtypes
    bf = ml_dtypes.bfloat16
    x = np.ascontiguousarray(x, dtype=np.float32)
    xt32 = np.ascontiguousarray(x.transpose(0, 2, 1))
    xqb = np.ascontiguousarray(xt32[:, :, 0:QL]).astype(bf)
    xtb = xt32.astype(bf)
    xnb = x.astype(bf)
    wqb = np.ascontiguousarray(np.asarray(Wq, dtype=np.float32).T).astype(bf)
    wkb = np.ascontiguousarray(np.asarray(Wk, dtype=np.float32)).astype(bf)
    wvb = np.ascontiguousarray(np.asarray(Wv, dtype=np.float32).T).astype(bf)
    wpb = np.ascontiguousarray(np.asarray(Wproj, dtype=np.float32).T).astype(bf)
    bpf = np.ascontiguousarray(np.asarray(bproj, dtype=np.float32).reshape(1, C))
    in_maps = []
    for core in range(8):
        s = slice(core * BL, (core + 1) * BL)
        in_maps.append({
            "xn": np.ascontiguousarray(xnb[s]),
            "xt": np.ascontiguousarray(xtb[s]),
            "xq": np.ascontiguousarray(xqb[s]),
            "wq": wqb, "wk": wkb, "wv": wvb, "wp": wpb, "bp": bpf,
        })
    return in_maps


def kernel(x, Wq, Wk, Wv, Wproj, bproj):
    from concourse import bass_utils
    nc = get_nc()
    in_maps = make_in_maps(x, Wq, Wk, Wv, Wproj, bproj)
    res = bass_utils.run_bass_kernel_spmd(nc, in_maps, core_ids=list(range(8)))
    out = np.concatenate([res.results[i]["y"] for i in range(8)], axis=0)
    return out.astype(np.float32)
